# revision 6
# baseline (speedup 1.0000x reference)
"""Brenier-map ICNN gradient kernel for Trainium2 (8 NeuronCores, data parallel).

Computes grad_u of sum(ICNN(u)) for the 5-layer input-convex network in the
reference: forward MLP with exp() weights + hand-derived backward pass.

Design:
  - Pure batch data-parallelism: each core gets 8192 of 65536 samples.
  - Host precomputes exp(weights), transposes, and bf16 casts.
  - On-chip layout keeps hidden units on partitions and samples on the free
    dim ("transposed" activations), so the z-chain (forward and backward)
    needs no transposes at all.  The gradient accumulation runs with the
    backward deltas as the *stationary* matmul operand, which produces the
    output in natural [samples, 64] layout directly.
  - All matmuls bf16 with fp32 PSUM accumulation.
  - LeakyReLU+bias is a single ACT-engine Prelu per tile (alpha=0.2); the
    derivative mask m = max(psum > -b, 0.2) is a single fused DVE
    tensor_scalar; backward applies it with one tensor_tensor per tile.
    Layer 0's combined factor a0*lrelu'(s0) is just Prelu(a0); its extra
    factor 2 is folded into the gradient-side copy of exp(wu0).
"""

import numpy as np
from contextlib import ExitStack

import concourse.bacc as bacc
import concourse.mybir as mybir
import concourse.tile as tile
from concourse.bass import ds
from concourse.bass_utils import run_bass_kernel_spmd
from ml_dtypes import bfloat16

B, D, H = 65536, 64, 512
N_CORES = 8
B_CORE = B // N_CORES        # 8192 samples per core
CHUNK = 512                  # samples per pipeline chunk
N_CHUNKS = B_CORE // CHUNK   # 16
NT = H // 128                # 4 hidden-dim tiles of 128
ALPHA = 0.2

F32 = mybir.dt.float32
BF16 = mybir.dt.bfloat16
AF = mybir.ActivationFunctionType
OP = mybir.AluOpType

_PROGRAMS = {}


def _body(ctx, tc, uT_d, euT_d, eu4T_d, ezT_d, ezn_d, ez4_d, eu4_d, eun_d,
          bias_d, negb_d, negb4_d, out_d):
    nc = tc.nc
    wpool = ctx.enter_context(tc.tile_pool(name="weights", bufs=1))
    acts = ctx.enter_context(tc.tile_pool(name="acts", bufs=2))
    dspool = ctx.enter_context(tc.tile_pool(name="dsp", bufs=3))
    iop = ctx.enter_context(tc.tile_pool(name="io", bufs=2))
    pps = ctx.enter_context(tc.tile_pool(name="pps", bufs=2, space="PSUM"))
    pps4 = ctx.enter_context(tc.tile_pool(name="pps4", bufs=1, space="PSUM"))
    pdz = ctx.enter_context(tc.tile_pool(name="pdz", bufs=2, space="PSUM"))
    pgu = ctx.enter_context(tc.tile_pool(name="pgu", bufs=2, space="PSUM"))

    # ---- resident inputs (loaded once) ----
    uT_s = wpool.tile([D, B_CORE], BF16)
    nc.sync.dma_start(out=uT_s, in_=uT_d)
    euT_s = wpool.tile([D, 4 * H], BF16)
    nc.sync.dma_start(out=euT_s, in_=euT_d)
    eu4T_s = wpool.tile([D, 1], BF16)
    nc.sync.dma_start(out=eu4T_s, in_=eu4T_d)
    ezT_s = wpool.tile([128, 3, NT, H], BF16)
    nc.sync.dma_start(out=ezT_s, in_=ezT_d.rearrange("i (k p) n -> p i k n", p=128))
    ezn_s = wpool.tile([128, 3, NT, H], BF16)
    nc.sync.dma_start(out=ezn_s, in_=ezn_d.rearrange("i (k p) n -> p i k n", p=128))
    ez4T_s = wpool.tile([128, NT], BF16)
    nc.sync.dma_start(out=ez4T_s, in_=ez4_d.rearrange("(k p) -> p k", p=128))
    ez4r_s = wpool.tile([1, H], BF16)
    nc.sync.dma_start(out=ez4r_s, in_=ez4_d.rearrange("(o h) -> o h", o=1))
    eu4_s = wpool.tile([1, D], BF16)
    nc.sync.dma_start(out=eu4_s, in_=eu4_d)
    eun_s = wpool.tile([128, 4 * NT, D], BF16)
    nc.sync.dma_start(out=eun_s, in_=eun_d.rearrange("b p d -> p b d"))
    bias_s = wpool.tile([128, 4, NT], F32)
    nc.sync.dma_start(out=bias_s, in_=bias_d.rearrange("i (j p) -> p i j", p=128))
    negb_s = wpool.tile([128, 4, NT], F32)
    nc.sync.dma_start(out=negb_s, in_=negb_d.rearrange("i (j p) -> p i j", p=128))
    negb4_s = wpool.tile([1, 1], F32)
    nc.sync.dma_start(out=negb4_s, in_=negb4_d)
    zeros_s = wpool.tile([1, NT * D], BF16)
    nc.vector.memset(zeros_s, 0.0)

    out_v = out_d.rearrange("(c g p) d -> c p g d", g=NT, p=128)

    for c in range(N_CHUNKS):
        cs = ds(c * CHUNK, CHUNK)

        # ---------------- forward ----------------
        # layer 0: z0 = lrelu(u @ E0.T + b0)^2; g0 = a0 * lrelu'(s0)
        z0 = acts.tile([128, NT, CHUNK], BF16, name="z0")
        g0 = acts.tile([128, NT, CHUNK], BF16, name="g0")
        for j in range(NT):
            sp = pps.tile([128, CHUNK], F32, name="sp")
            nc.tensor.matmul(sp, euT_s[:, ds(j * 128, 128)], uT_s[:, cs],
                             start=True, stop=True)
            a0 = acts.tile([128, CHUNK], BF16, name="a0")
            nc.scalar.activation(a0, sp, AF.Prelu,
                                 bias=bias_s[:, 0, j:j + 1], alpha=ALPHA)
            nc.scalar.square(z0[:, j, :], a0)
            nc.scalar.activation(g0[:, j, :], a0, AF.Prelu, alpha=ALPHA)

        # layers 1..3: z_i = lrelu(u @ Eu_i.T + z_{i-1} @ Ez_i.T + b_i)
        zp = z0
        ms = {}
        for i in (1, 2, 3):
            zi = acts.tile([128, NT, CHUNK], BF16, name=f"z{i}")
            mi = acts.tile([128, NT, CHUNK], BF16, name=f"m{i}")
            for j in range(NT):
                sp = pps.tile([128, CHUNK], F32, name="sp")
                nc.tensor.matmul(sp, euT_s[:, ds(i * H + j * 128, 128)],
                                 uT_s[:, cs], start=True, stop=False)
                for k in range(NT):
                    nc.tensor.matmul(sp, ezT_s[:, i - 1, k, ds(j * 128, 128)],
                                     zp[:, k, :], start=False, stop=(k == NT - 1))
                nc.vector.tensor_scalar(mi[:, j, :], sp, negb_s[:, i, j:j + 1],
                                        ALPHA, OP.is_gt, OP.max)
                nc.scalar.activation(zi[:, j, :], sp, AF.Prelu,
                                     bias=bias_s[:, i, j:j + 1], alpha=ALPHA)
            zp = zi
            ms[i] = mi

        # layer 4 (scalar head): only the lrelu' mask ds4 is needed
        s4p = pps4.tile([1, CHUNK], F32, name="s4p")
        nc.tensor.matmul(s4p, eu4T_s, uT_s[:, cs], start=True, stop=False)
        for k in range(NT):
            nc.tensor.matmul(s4p, ez4T_s[:, k:k + 1], zp[:, k, :],
                             start=False, stop=(k == NT - 1))
        ds4 = dspool.tile([1, CHUNK], BF16, name="ds4")
        nc.vector.tensor_scalar(ds4, s4p, negb4_s, ALPHA, OP.is_gt, OP.max)

        # ---------------- backward ----------------
        # grad accumulator in natural [samples, 64] layout; backward deltas
        # are the stationary operand so no output transpose is needed.
        gup = pgu.tile([128, NT, D], F32, name="gup")
        # single accumulation group over the whole bank: zero it with one
        # K=1 matmul (start=True), then everything accumulates into it.
        nc.tensor.matmul(gup[:, :, :], zeros_s[:, 0:128], zeros_s,
                         start=True, stop=False)
        for g in range(NT):
            nc.tensor.matmul(gup[:, g, :], ds4[:, ds(g * 128, 128)], eu4_s,
                             start=False, stop=False)

        # dz3 = outer(Ez4, ds4);  ds3 = dz3 * m3
        dst = {}
        for j in range(NT):
            dzp = pdz.tile([128, CHUNK], F32, name="dzp")
            nc.tensor.matmul(dzp, ez4r_s[:, ds(j * 128, 128)], ds4,
                             start=True, stop=True)
            dd = dspool.tile([128, CHUNK], BF16, name=f"ds3_{j}")
            nc.vector.tensor_tensor(dd, dzp, ms[3][:, j, :], OP.mult)
            dst[j] = dd

        for i in (3, 2, 1):
            # gu += ds_i @ Eu_i
            for j in range(NT):
                for g in range(NT):
                    nc.tensor.matmul(gup[:, g, :], dst[j][:, ds(g * 128, 128)],
                                     eun_s[:, i * NT + j, :],
                                     start=False, stop=False)
            # dz_{i-1} = ds_i @ Ez_i ; ds_{i-1} = dz * m_{i-1} (g0 for i==1)
            nxt = {}
            for j in range(NT):
                dzp = pdz.tile([128, CHUNK], F32, name="dzp")
                for k in range(NT):
                    nc.tensor.matmul(dzp, ezn_s[:, i - 1, k, ds(j * 128, 128)],
                                     dst[k], start=(k == 0), stop=(k == NT - 1))
                dd = dspool.tile([128, CHUNK], BF16, name=f"ds_{j}")
                mul = g0[:, j, :] if i == 1 else ms[i - 1][:, j, :]
                nc.vector.tensor_tensor(dd, dzp, mul, OP.mult)
                nxt[j] = dd
            dst = nxt

        # gu += ds0 @ (2*E0)  (factor 2 folded into eun block 0 on the host)
        for j in range(NT):
            for g in range(NT):
                nc.tensor.matmul(gup[:, g, :], dst[j][:, ds(g * 128, 128)],
                                 eun_s[:, j, :], start=False,
                                 stop=(j == NT - 1 and g == NT - 1))

        gsb = iop.tile([128, NT, D], F32, name="gsb")
        nc.scalar.copy(gsb, gup)
        nc.sync.dma_start(out=out_v[c], in_=gsb)


def _build_program():
    nc = bacc.Bacc("TRN2", target_bir_lowering=False, debug=False,
                   enable_asserts=False)
    uT_d = nc.dram_tensor("uT", [D, B_CORE], BF16, kind="ExternalInput").ap()
    euT_d = nc.dram_tensor("euT", [D, 4 * H], BF16, kind="ExternalInput").ap()
    eu4T_d = nc.dram_tensor("eu4T", [D, 1], BF16, kind="ExternalInput").ap()
    ezT_d = nc.dram_tensor("ezT", [3, H, H], BF16, kind="ExternalInput").ap()
    ezn_d = nc.dram_tensor("ezn", [3, H, H], BF16, kind="ExternalInput").ap()
    ez4_d = nc.dram_tensor("ez4", [H], BF16, kind="ExternalInput").ap()
    eu4_d = nc.dram_tensor("eu4", [1, D], BF16, kind="ExternalInput").ap()
    eun_d = nc.dram_tensor("eun", [4 * NT, 128, D], BF16, kind="ExternalInput").ap()
    bias_d = nc.dram_tensor("bias", [4, H], F32, kind="ExternalInput").ap()
    negb_d = nc.dram_tensor("negb", [4, H], F32, kind="ExternalInput").ap()
    negb4_d = nc.dram_tensor("negb4", [1, 1], F32, kind="ExternalInput").ap()
    out_d = nc.dram_tensor("out", [B_CORE, D], F32, kind="ExternalOutput").ap()

    with ExitStack() as ctx:
        tc = ctx.enter_context(tile.TileContext(nc))
        _body(ctx, tc, uT_d, euT_d, eu4T_d, ezT_d, ezn_d, ez4_d, eu4_d, eun_d,
              bias_d, negb_d, negb4_d, out_d)
    nc.compile()
    return nc


def _get_program():
    if "main" not in _PROGRAMS:
        _PROGRAMS["main"] = _build_program()
    return _PROGRAMS["main"]


def _prepare_in_maps(inputs):
    u = np.asarray(inputs["u"], dtype=np.float32)
    wu = [np.asarray(inputs[f"wu{i}"], np.float32) for i in range(5)]
    wz = {i: np.asarray(inputs[f"wz{i}"], np.float32) for i in (1, 2, 3, 4)}
    b = [np.asarray(inputs[f"b{i}"], np.float32) for i in range(5)]

    Eu = [np.exp(w) for w in wu]           # [H, D]; Eu[4] is [1, D]
    Ez = {i: np.exp(wz[i]) for i in wz}    # [H, H]; Ez[4] is [1, H]

    euT = np.concatenate([Eu[i].T for i in range(4)], axis=1)      # [D, 4H]
    bias = np.stack([b[i] for i in range(4)])                      # [4, H]

    bf = lambda x: np.ascontiguousarray(x, dtype=np.float32).astype(bfloat16)
    f32 = lambda x: np.ascontiguousarray(x, dtype=np.float32)
    weights = {
        "euT": bf(euT),
        "eu4T": bf(Eu[4].T),
        "ezT": bf(np.stack([Ez[i].T for i in (1, 2, 3)])),
        "ezn": bf(np.stack([Ez[i] for i in (1, 2, 3)])),
        "ez4": bf(Ez[4][0]),
        "eu4": bf(Eu[4]),
        "eun": bf(np.concatenate([2.0 * Eu[0], Eu[1], Eu[2], Eu[3]],
                                 axis=0).reshape(4 * NT, 128, D)),
        "bias": f32(bias),
        "negb": f32(-bias),
        "negb4": f32(-b[4].reshape(1, 1)),
    }

    in_maps = []
    for core in range(N_CORES):
        ush = u[core * B_CORE:(core + 1) * B_CORE]
        in_maps.append({"uT": bf(ush.T), **weights})
    return in_maps


def kernel(**inputs):
    in_maps = _prepare_in_maps(inputs)
    nc = _get_program()
    res = run_bass_kernel_spmd(nc, in_maps, core_ids=list(range(N_CORES)))
    return np.concatenate([res.results[i]["out"] for i in range(N_CORES)],
                          axis=0)


# revision 10
# speedup vs baseline: 2199.8091x; 2199.8091x over previous
"""Brenier-map ICNN gradient kernel for Trainium2 (8 NeuronCores, data parallel).

Computes grad_u of sum(ICNN(u)) for the 5-layer input-convex network in the
reference: forward MLP with exp() weights + hand-derived backward pass.

Design:
  - Pure batch data-parallelism: each core gets 8192 of 65536 samples.
  - Host precomputes exp(weights), transposes, and bf16 casts.
  - On-chip layout keeps hidden units on partitions and samples on the free
    dim ("transposed" activations), so the z-chain (forward and backward)
    needs no transposes at all.  The gradient accumulation runs with the
    backward deltas as the *stationary* matmul operand, which produces the
    output in natural [samples, 64] layout directly.
  - All matmuls bf16 with fp32 PSUM accumulation.
  - LeakyReLU+bias is a single ACT-engine Prelu per tile (alpha=0.2); the
    derivative mask m = max(psum > -b, 0.2) is a single fused DVE
    tensor_scalar; backward applies it with one tensor_tensor per tile.
    Layer 0's combined factor a0*lrelu'(s0) is just Prelu(a0); its extra
    factor 2 is folded into the gradient-side copy of exp(wu0).
"""

import numpy as np
from contextlib import ExitStack

import concourse.bacc as bacc
import concourse.mybir as mybir
import concourse.tile as tile
from concourse.bass import ds
from concourse.bass_utils import run_bass_kernel_spmd
from ml_dtypes import bfloat16

B, D, H = 65536, 64, 512
N_CORES = 8
B_CORE = B // N_CORES        # 8192 samples per core
CHUNK = 512                  # samples per pipeline chunk
N_CHUNKS = B_CORE // CHUNK   # 16
NT = H // 128                # 4 hidden-dim tiles of 128
ALPHA = 0.2

F32 = mybir.dt.float32
BF16 = mybir.dt.bfloat16
AF = mybir.ActivationFunctionType
OP = mybir.AluOpType

_PROGRAMS = {}


def _body(ctx, tc, uT_d, euT_d, eu4T_d, ezT_d, ezn_d, ez4_d, eu4_d, eun_d,
          bias_d, negb_d, negb4_d, out_d):
    nc = tc.nc
    wpool = ctx.enter_context(tc.tile_pool(name="weights", bufs=1))
    acts = ctx.enter_context(tc.tile_pool(name="acts", bufs=2))
    dspool = ctx.enter_context(tc.tile_pool(name="dsp", bufs=3))
    iop = ctx.enter_context(tc.tile_pool(name="io", bufs=2))
    utp = ctx.enter_context(tc.tile_pool(name="utp", bufs=3))
    pps = ctx.enter_context(tc.tile_pool(name="pps", bufs=3, space="PSUM"))
    pps4 = ctx.enter_context(tc.tile_pool(name="pps4", bufs=1, space="PSUM"))
    pdz = ctx.enter_context(tc.tile_pool(name="pdz", bufs=2, space="PSUM"))
    pgu = ctx.enter_context(tc.tile_pool(name="pgu", bufs=2, space="PSUM"))

    # ---- resident inputs (loaded once; uT streams per chunk) ----
    euT_s = wpool.tile([D, 4 * H], BF16)
    nc.sync.dma_start(out=euT_s, in_=euT_d)
    eu4T_s = wpool.tile([D, 1], BF16)
    nc.sync.dma_start(out=eu4T_s, in_=eu4T_d)
    ezT_s = wpool.tile([128, 3, NT, H], BF16)
    nc.sync.dma_start(out=ezT_s, in_=ezT_d.rearrange("i (k p) n -> p i k n", p=128))
    ezn_s = wpool.tile([128, 3, NT, H], BF16)
    nc.sync.dma_start(out=ezn_s, in_=ezn_d.rearrange("i (k p) n -> p i k n", p=128))
    ez4T_s = wpool.tile([128, NT], BF16)
    nc.sync.dma_start(out=ez4T_s, in_=ez4_d.rearrange("(k p) -> p k", p=128))
    ez4r_s = wpool.tile([1, H], BF16)
    nc.sync.dma_start(out=ez4r_s, in_=ez4_d.rearrange("(o h) -> o h", o=1))
    eu4_s = wpool.tile([1, D], BF16)
    nc.sync.dma_start(out=eu4_s, in_=eu4_d)
    eun_s = wpool.tile([128, 4 * NT, D], BF16)
    nc.sync.dma_start(out=eun_s, in_=eun_d.rearrange("b p d -> p b d"))
    bias_s = wpool.tile([128, 4, NT], F32)
    nc.sync.dma_start(out=bias_s, in_=bias_d.rearrange("i (j p) -> p i j", p=128))
    negb_s = wpool.tile([128, 4, NT], F32)
    nc.sync.dma_start(out=negb_s, in_=negb_d.rearrange("i (j p) -> p i j", p=128))
    negb4_s = wpool.tile([1, 1], F32)
    nc.sync.dma_start(out=negb4_s, in_=negb4_d)
    zeros_s = wpool.tile([1, NT * D], BF16)
    nc.vector.memset(zeros_s, 0.0)

    out_v = out_d.rearrange("(c g p) d -> c p g d", g=NT, p=128)

    for c in range(N_CHUNKS):
        cs = ds(c * CHUNK, CHUNK)
        ut = utp.tile([D, CHUNK], BF16, name="ut")
        nc.sync.dma_start(out=ut, in_=uT_d[:, cs])

        # ---------------- forward ----------------
        # layer 0: z0 = lrelu(u @ E0.T + b0)^2; g0 = a0 * lrelu'(s0)
        z0 = acts.tile([128, NT, CHUNK], BF16, name="z0")
        g0 = acts.tile([128, NT, CHUNK], BF16, name="g0")
        for j in range(NT):
            sp = pps.tile([128, CHUNK], F32, name="sp")
            nc.tensor.matmul(sp, euT_s[:, ds(j * 128, 128)], ut,
                             start=True, stop=True)
            a0 = acts.tile([128, CHUNK], BF16, name="a0")
            nc.scalar.activation(a0, sp, AF.Prelu,
                                 bias=bias_s[:, 0, j:j + 1], alpha=ALPHA)
            nc.scalar.square(z0[:, j, :], a0)
            nc.scalar.activation(g0[:, j, :], a0, AF.Prelu, alpha=ALPHA)

        # layers 1..3: z_i = lrelu(u @ Eu_i.T + z_{i-1} @ Ez_i.T + b_i)
        zp = z0
        ms = {}
        for i in (1, 2, 3):
            zi = acts.tile([128, NT, CHUNK], BF16, name=f"z{i}")
            mi = acts.tile([128, NT, CHUNK], BF16, name=f"m{i}")
            for j in range(NT):
                sp = pps.tile([128, CHUNK], F32, name="sp")
                nc.tensor.matmul(sp, euT_s[:, ds(i * H + j * 128, 128)],
                                 ut, start=True, stop=False)
                for k in range(NT):
                    nc.tensor.matmul(sp, ezT_s[:, i - 1, k, ds(j * 128, 128)],
                                     zp[:, k, :], start=False, stop=(k == NT - 1))
                nc.vector.tensor_scalar(mi[:, j, :], sp, negb_s[:, i, j:j + 1],
                                        ALPHA, OP.is_gt, OP.max)
                nc.scalar.activation(zi[:, j, :], sp, AF.Prelu,
                                     bias=bias_s[:, i, j:j + 1], alpha=ALPHA)
            zp = zi
            ms[i] = mi

        # layer 4 (scalar head): only the lrelu' mask ds4 is needed
        s4p = pps4.tile([1, CHUNK], F32, name="s4p")
        nc.tensor.matmul(s4p, eu4T_s, ut, start=True, stop=False)
        for k in range(NT):
            nc.tensor.matmul(s4p, ez4T_s[:, k:k + 1], zp[:, k, :],
                             start=False, stop=(k == NT - 1))
        ds4 = dspool.tile([1, CHUNK], BF16, name="ds4")
        nc.vector.tensor_scalar(ds4, s4p, negb4_s, ALPHA, OP.is_gt, OP.max)

        # ---------------- backward ----------------
        # grad accumulator in natural [samples, 64] layout; backward deltas
        # are the stationary operand so no output transpose is needed.
        gup = pgu.tile([128, NT, D], F32, name="gup")
        # single accumulation group over the whole bank: zero it with one
        # K=1 matmul (start=True), then everything accumulates into it.
        nc.tensor.matmul(gup[:, :, :], zeros_s[:, 0:128], zeros_s,
                         start=True, stop=False)
        for g in range(NT):
            nc.tensor.matmul(gup[:, g, :], ds4[:, ds(g * 128, 128)], eu4_s,
                             start=False, stop=False)

        # dz3 = outer(Ez4, ds4);  ds3 = dz3 * m3
        dst = {}
        for j in range(NT):
            dzp = pdz.tile([128, CHUNK], F32, name="dzp")
            nc.tensor.matmul(dzp, ez4r_s[:, ds(j * 128, 128)], ds4,
                             start=True, stop=True)
            dd = dspool.tile([128, CHUNK], BF16, name=f"ds3_{j}")
            nc.vector.tensor_tensor(dd, dzp, ms[3][:, j, :], OP.mult)
            dst[j] = dd

        for i in (3, 2, 1):
            # gu += ds_i @ Eu_i
            for j in range(NT):
                for g in range(NT):
                    nc.tensor.matmul(gup[:, g, :], dst[j][:, ds(g * 128, 128)],
                                     eun_s[:, i * NT + j, :],
                                     start=False, stop=False)
            # dz_{i-1} = ds_i @ Ez_i ; ds_{i-1} = dz * m_{i-1} (g0 for i==1)
            nxt = {}
            for j in range(NT):
                dzp = pdz.tile([128, CHUNK], F32, name="dzp")
                for k in range(NT):
                    nc.tensor.matmul(dzp, ezn_s[:, i - 1, k, ds(j * 128, 128)],
                                     dst[k], start=(k == 0), stop=(k == NT - 1))
                dd = dspool.tile([128, CHUNK], BF16, name=f"ds_{j}")
                mul = g0[:, j, :] if i == 1 else ms[i - 1][:, j, :]
                nc.vector.tensor_tensor(dd, dzp, mul, OP.mult)
                nxt[j] = dd
            dst = nxt

        # gu += ds0 @ (2*E0)  (factor 2 folded into eun block 0 on the host)
        for j in range(NT):
            for g in range(NT):
                nc.tensor.matmul(gup[:, g, :], dst[j][:, ds(g * 128, 128)],
                                 eun_s[:, j, :], start=False,
                                 stop=(j == NT - 1 and g == NT - 1))

        gsb = iop.tile([128, NT, D], F32, name="gsb")
        nc.scalar.copy(gsb, gup)
        nc.sync.dma_start(out=out_v[c], in_=gsb)


def _build_program():
    nc = bacc.Bacc("TRN2", target_bir_lowering=False, debug=False,
                   enable_asserts=False)
    uT_d = nc.dram_tensor("uT", [D, B_CORE], BF16, kind="ExternalInput").ap()
    euT_d = nc.dram_tensor("euT", [D, 4 * H], BF16, kind="ExternalInput").ap()
    eu4T_d = nc.dram_tensor("eu4T", [D, 1], BF16, kind="ExternalInput").ap()
    ezT_d = nc.dram_tensor("ezT", [3, H, H], BF16, kind="ExternalInput").ap()
    ezn_d = nc.dram_tensor("ezn", [3, H, H], BF16, kind="ExternalInput").ap()
    ez4_d = nc.dram_tensor("ez4", [H], BF16, kind="ExternalInput").ap()
    eu4_d = nc.dram_tensor("eu4", [1, D], BF16, kind="ExternalInput").ap()
    eun_d = nc.dram_tensor("eun", [4 * NT, 128, D], BF16, kind="ExternalInput").ap()
    bias_d = nc.dram_tensor("bias", [4, H], F32, kind="ExternalInput").ap()
    negb_d = nc.dram_tensor("negb", [4, H], F32, kind="ExternalInput").ap()
    negb4_d = nc.dram_tensor("negb4", [1, 1], F32, kind="ExternalInput").ap()
    out_d = nc.dram_tensor("out", [B_CORE, D], F32, kind="ExternalOutput").ap()

    with ExitStack() as ctx:
        tc = ctx.enter_context(tile.TileContext(nc))
        _body(ctx, tc, uT_d, euT_d, eu4T_d, ezT_d, ezn_d, ez4_d, eu4_d, eun_d,
              bias_d, negb_d, negb4_d, out_d)
    nc.compile()
    return nc


def _get_program():
    if "main" not in _PROGRAMS:
        _PROGRAMS["main"] = _build_program()
    return _PROGRAMS["main"]


def _prepare_in_maps(inputs):
    u = np.asarray(inputs["u"], dtype=np.float32)
    wu = [np.asarray(inputs[f"wu{i}"], np.float32) for i in range(5)]
    wz = {i: np.asarray(inputs[f"wz{i}"], np.float32) for i in (1, 2, 3, 4)}
    b = [np.asarray(inputs[f"b{i}"], np.float32) for i in range(5)]

    Eu = [np.exp(w) for w in wu]           # [H, D]; Eu[4] is [1, D]
    Ez = {i: np.exp(wz[i]) for i in wz}    # [H, H]; Ez[4] is [1, H]

    euT = np.concatenate([Eu[i].T for i in range(4)], axis=1)      # [D, 4H]
    bias = np.stack([b[i] for i in range(4)])                      # [4, H]

    bf = lambda x: np.ascontiguousarray(x, dtype=np.float32).astype(bfloat16)
    f32 = lambda x: np.ascontiguousarray(x, dtype=np.float32)
    weights = {
        "euT": bf(euT),
        "eu4T": bf(Eu[4].T),
        "ezT": bf(np.stack([Ez[i].T for i in (1, 2, 3)])),
        "ezn": bf(np.stack([Ez[i] for i in (1, 2, 3)])),
        "ez4": bf(Ez[4][0]),
        "eu4": bf(Eu[4]),
        "eun": bf(np.concatenate([2.0 * Eu[0], Eu[1], Eu[2], Eu[3]],
                                 axis=0).reshape(4 * NT, 128, D)),
        "bias": f32(bias),
        "negb": f32(-bias),
        "negb4": f32(-b[4].reshape(1, 1)),
    }

    in_maps = []
    for core in range(N_CORES):
        ush = u[core * B_CORE:(core + 1) * B_CORE]
        in_maps.append({"uT": bf(ush.T), **weights})
    return in_maps


def kernel(**inputs):
    in_maps = _prepare_in_maps(inputs)
    nc = _get_program()
    res = run_bass_kernel_spmd(nc, in_maps, core_ids=list(range(N_CORES)))
    return np.concatenate([res.results[i]["out"] for i in range(N_CORES)],
                          axis=0)
